# revision 103
# baseline (speedup 1.0000x reference)
"""Trainium2 Bass kernel for nn_MultiHeadAttention_75548474736720.

Linear-attention-style multi-head attention with causal prefix sums:
  qh/kh/vh = projections, ph = split_heads(p)
  A1 = elu(qh ph^T) + 1                       [t,s] per (b,h)
  U  = (tril(qh kh^T)/idx) @ A1 ; W = softmax(U)
  S2 = (tril(W A1^T)/idx) ; out = (S2 @ vh) reshaped @ wc + b

Sharding: 8 cores = (batch b in 0..1) x (head-group hg in 0..3, 4 heads each).
Each core computes its 4 heads end-to-end (wq/wk/wv column-sliced, wc
row-sliced) and returns a partial [S, Dm] output; host sums partials per batch.

Most matmuls run in bf16 (f32 PSUM accumulation); the S2 = W @ A1^T
contraction runs in fp8 with DoubleRow perf mode (2 k-tiles per
instruction at 0.5 cycles/row -> 4x the bf16 rate).  Measured end-to-end
error vs the f32 reference is ~1.4e-2 (gate 2e-2).  Key tricks:
  - W = exp(U/(t+1) - 10) cast to fp8e5: the constant shift keeps exp in
    e5m2 range (U/(t+1) in [-19, 19]) and cancels between S2 and den, so
    no per-row max pass is needed; A1^T cast to fp8e4 (A1 <= ~8)
  - softmax denominator via ACT accum_out (free with the exp pass, f32)
  - per-row 1/(t+1) scales folded into ACT scale APs (pre-exp and final)
  - W^T / A1^T via DMA-engine transposes (InstDmaTransposeAnt) + a DVE
    cast copy: no PE or ACT time spent on transposition
  - per-head generator emits A1[m] -> SqT[m] -> U[m] interleaved, pulled
    from inside head h-1's S2 phase: U is PE-heavy while S2 is DVE-heavy,
    so cross-head software pipelining keeps both engines fed
  - S2 emitted column-major matching wtT production order, with the O
    chains and (last pair) the output projection inlined per column
"""

import sys

sys.path.insert(0, "/opt/trn_rl_repo")

import ml_dtypes
import numpy as np

import concourse.bass as bass  # noqa: F401  (registers AP machinery)
import concourse.mybir as mybir
from concourse import bacc
from concourse.tile import TileContext
from concourse.bass_utils import run_bass_kernel_spmd

F32 = mybir.dt.float32
BF16 = mybir.dt.bfloat16
FP8 = mybir.dt.float8e4
FP8W = mybir.dt.float8e5
WSHIFT = -10.0
ACTF = mybir.ActivationFunctionType
ALU = mybir.AluOpType
AXL = mybir.AxisListType
DROW = mybir.MatmulPerfMode.DoubleRow
NPBF = ml_dtypes.bfloat16
LN16 = float(np.log(16.0))

B, S, DM, H = 2, 1024, 1024, 16
D = DM // H            # 64, head dim
HG = 4                 # heads per core
DL = HG * D            # 256, local dm slice
NB = S // 128          # 8 s-blocks
NORM_D = 0.125         # 1/sqrt(D)

# compact SqT layout: per s-block m, columns stored from t = 512*(m//4)
SQBASE = [0, 1024, 2048, 3072, 4096, 4608, 5120, 5632]  # total 6144

DEBUG = False


def _sq_off(m, t0):
    return SQBASE[m] + t0 - 512 * (m // 4)


def _build_program():
    nc = bacc.Bacc(None, target_bir_lowering=False)

    qT_in = nc.declare_dram_parameter("qT", [DM, S], BF16, isOutput=False)
    kT_in = nc.declare_dram_parameter("kT", [DM, S], BF16, isOutput=False)
    vT_in = nc.declare_dram_parameter("vT", [DM, S], BF16, isOutput=False)
    pT_in = nc.declare_dram_parameter("pT", [DL, S], BF16, isOutput=False)
    wq_in = nc.declare_dram_parameter("wq", [DM, DL], BF16, isOutput=False)
    wk_in = nc.declare_dram_parameter("wk", [DM, DL], BF16, isOutput=False)
    wv_in = nc.declare_dram_parameter("wv", [DM, DL], BF16, isOutput=False)
    wc_in = nc.declare_dram_parameter("wc", [DL, S], BF16, isOutput=False)
    wqb_in = nc.declare_dram_parameter("wqb", [128, 2], F32, isOutput=False)
    wkb_in = nc.declare_dram_parameter("wkb", [128, 2], F32, isOutput=False)
    wvb_in = nc.declare_dram_parameter("wvb", [1, DL], BF16, isOutput=False)
    ones_in = nc.declare_dram_parameter("ones1", [1, 128], BF16, isOutput=False)
    mask_in = nc.declare_dram_parameter("mask4", [4, 128, 512], BF16, isOutput=False)
    ident_in = nc.declare_dram_parameter("ident", [128, 128], BF16, isOutput=False)
    ident8_in = nc.declare_dram_parameter("ident8", [128, 128], FP8, isOutput=False)
    inv_in = nc.declare_dram_parameter("invidx", [128, NB], F32, isOutput=False)
    out_d = nc.declare_dram_parameter("out", [S, DM], F32, isOutput=True)
    dbg = {}
    if DEBUG:
        dbg["qhT"] = nc.declare_dram_parameter("d_qhT", [128, 2 * S], F32, isOutput=True)
        dbg["vh"] = nc.declare_dram_parameter("d_vh", [128, NB * DL], F32, isOutput=True)
        dbg["a1"] = nc.declare_dram_parameter("d_a1", [128, NB * S], F32, isOutput=True)
        dbg["sqT"] = nc.declare_dram_parameter("d_sqT", [128, 6144], F32, isOutput=True)
        dbg["wtT"] = nc.declare_dram_parameter("d_wtT", [128, NB * S], F32, isOutput=True)
        dbg["oT"] = nc.declare_dram_parameter("d_oT", [64, HG * S], F32, isOutput=True)
        dbg["den"] = nc.declare_dram_parameter("d_den", [128, NB], F32, isOutput=True)

    with TileContext(nc) as tc:
        with tc.tile_pool(name="persist", bufs=1) as cp, \
             tc.tile_pool(name="ppm", bufs=6, space="PSUM") as ppm:

            mask = cp.tile([128, 4, 512], BF16)
            ident = cp.tile([128, 128], BF16)
            ident8 = cp.tile([128, 128], FP8)
            invidx = cp.tile([128, NB], F32)
            neginv = cp.tile([128, NB], F32)
            wsh = cp.tile([128, 1], F32)
            dumt = cp.tile([128, 1], F32)
            wqb = cp.tile([128, 2], F32)
            wkb = cp.tile([128, 2], F32)
            wvb = cp.tile([1, DL], BF16)
            ones1 = cp.tile([1, 128], BF16)
            pTt = cp.tile([128, 2, S], BF16)
            qhT = cp.tile([128, 2, S], BF16)
            khT = cp.tile([128, 2, S], BF16)
            vh = cp.tile([128, NB, DL], BF16)
            oT = cp.tile([128, 2, S], BF16)
            # wc stored per head-pair: wct[:, g, :] = wc rows [g*128:(g+1)*128];
            # loaded up front so the output projection never waits on DMA
            wct = cp.tile([128, 2, S], BF16)

            # ---------------- projections ----------------
            # DMA issue on SP costs ~0.5us per descriptor, so the inputs the
            # first matmuls need go first, split 4-ways for queue parallelism;
            # constants (masks, wc, p) follow.  The v projection runs inside
            # the attention phase (interleaved with head 0's A1/SqT) so its
            # tiles live in a separate pool that outlives the q/k one.
            vp_cm = tc.tile_pool(name="vproj", bufs=1)
            vp = vp_cm.__enter__()
            wvt = vp.tile([128, NB, DL], BF16)
            vTt = vp.tile([128, NB, S], BF16)
            with tc.tile_pool(name="proj", bufs=1) as jp:
                wqt = jp.tile([128, NB, DL], BF16)
                wkt = jp.tile([128, NB, DL], BF16)
                qTt = jp.tile([128, NB, S], BF16)
                kTt = jp.tile([128, NB, S], BF16)
                for wt_, wsrc, xt_, xsrc in ((wqt, wq_in, qTt, qT_in),
                                             (wkt, wk_in, kTt, kT_in),
                                             (wvt, wv_in, vTt, vT_in)):
                    for q4 in range(4):
                        kb = 2 * q4
                        nc.sync.dma_start(
                            out=wt_[:, kb:kb + 2, :],
                            in_=wsrc[kb * 128:(kb + 2) * 128, :].rearrange(
                                "(a p) d -> p a d", p=128))
                        nc.sync.dma_start(
                            out=xt_[:, kb:kb + 2, :],
                            in_=xsrc[kb * 128:(kb + 2) * 128, :].rearrange(
                                "(a p) t -> p a t", p=128))
                    if wt_ is wqt:
                        nc.sync.dma_start(
                            out=pTt[:], in_=pT_in.rearrange("(g p) t -> p g t", p=128))
                        nc.sync.dma_start(out=wqb[:], in_=wqb_in[:])
                        nc.sync.dma_start(out=invidx[:], in_=inv_in[:])
                        nc.vector.tensor_scalar_mul(neginv[:], invidx[:], -1.0)
                        nc.gpsimd.memset(wsh[:], WSHIFT)
                        # dummy exp: pulls the ACT function-table load off the
                        # critical path (it otherwise serializes before the
                        # first real activation ~12us in)
                        nc.scalar.activation(dumt[:], wsh[:], ACTF.Exp)
                    elif wt_ is wkt:
                        nc.sync.dma_start(
                            out=mask[:], in_=mask_in.rearrange("r p c -> p r c"))
                        nc.sync.dma_start(out=ident[:], in_=ident_in[:])
                        nc.sync.dma_start(out=ident8[:], in_=ident8_in[:])
                        nc.sync.dma_start(out=wkb[:], in_=wkb_in[:])
                    else:
                        nc.sync.dma_start(out=wvb[:], in_=wvb_in[:])
                        nc.sync.dma_start(out=ones1[:], in_=ones_in[:])
                        nc.sync.dma_start(
                            out=wct[:], in_=wc_in.rearrange("(a p) t -> p a t", p=128))

                # qhT[dm, t] = sum_c wq[c, dm] qT[c, t]  (+bias, * 1/sqrt(D))
                # streamed: the kb loop is outermost so each accumulation step
                # depends only on the kb-block DMAs, overlapping PE with the
                # input load instead of waiting for the full tensor
                for wt_, xt_, dst, bias_t, scale in (
                    (wqt, qTt, qhT, wqb, NORM_D),
                    (wkt, kTt, khT, wkb, 1.0),
                ):
                    pss = [ppm.tile([128, 512], F32, tag="mm", name="ps_proj")
                           for _ in range(4)]
                    for kb in range(NB):
                        for g in range(2):
                            for n in range(2):
                                nc.tensor.matmul(
                                    pss[2 * g + n][:],
                                    wt_[:, kb, g * 128:(g + 1) * 128],
                                    xt_[:, kb, n * 512:(n + 1) * 512],
                                    start=(kb == 0), stop=(kb == NB - 1))
                    for g in range(2):
                        for n in range(2):
                            nc.scalar.activation(
                                dst[:, g, n * 512:(n + 1) * 512], pss[2 * g + n][:],
                                ACTF.Identity, bias=bias_t[:, g:g + 1], scale=scale)

                if DEBUG:
                    nc.sync.dma_start(out=dbg["qhT"].rearrange("p (a b) -> p a b", a=2),
                                      in_=qhT[:])

            # ---------------- attention (4 heads) ----------------
            # Pair-level software pipeline: A1/SqT for head h+1 are emitted
            # between U(h) and S2(h) so the in-order PE stream always has
            # independent matmuls to run while elementwise chains drain.
            with tc.tile_pool(name="attn", bufs=2) as ap, \
                 tc.tile_pool(name="scr", bufs=2) as sp:
                st = {}

                def gen_head(h):
                    """Generator: per m-block emits A1[m], SqT[m], then U[i=m]
                    (whose inputs are complete), yielding after each unit so
                    head h's whole front end can interleave into head h-1's
                    S2 phase: S2 is DVE/DMA-heavy while U is PE-heavy, so
                    overlapping them keeps both engines fed.

                    A1 = elu(x)+1 = min(exp(x), 1) + relu(x); exp is safe
                    unclamped (|x| <= ~8 here).  W = exp(U/(t+1) - 10) in
                    bf16 -> DMA-transposed -> cast to fp8e5; the constant
                    shift keeps exp within fp8e5 range (U/(t+1) <= ~19) and
                    cancels between S2 and den so gsc is unchanged."""
                    g, p0 = h // 2, (h % 2) * 64
                    a1 = ap.tile([128, NB, S], BF16, tag="a1", name="a1")
                    sqT = ap.tile([128, 6144], BF16, tag="sq", name="sqT")
                    wtT = ap.tile([128, NB, S], FP8W, tag="wtT", bufs=2, name="wtT")
                    a1Ts = {}
                    st[h] = [a1, sqT, wtT, a1Ts]
                    wblks = []

                    def emit_w_transpose(i):
                        wblk = wblks[i]
                        wtTb = sp.tile([128, NB, 128], BF16, tag="wtTb", bufs=3,
                                       name="wtTb")
                        nc.sync.dma_start(out=wtTb[:], in_=wblk[:], transpose=True)
                        nc.vector.tensor_copy(
                            wtT[:, :, i * 128:(i + 1) * 128], wtTb[:])

                    def emit_a1t(m):
                        a1Tb = sp.tile([128, NB, 128], BF16, tag="a1Tb", bufs=4,
                                       name="a1Tb")
                        a1T = sp.tile([128, NB, 128], FP8, tag="a1T", bufs=8,
                                      name="a1T")
                        a1Ts[m] = a1T
                        nc.sync.dma_start(out=a1Tb[:], in_=a1[:, m, :],
                                          transpose=True)
                        nc.vector.tensor_copy(a1T[:], a1Tb[:])

                    st[h].append(emit_a1t)
                    denB = sp.tile([128, NB], F32, tag="denB", name="denB")
                    for m in range(NB):
                        for c in range(2):
                            ps = ppm.tile([128, 512], F32, tag="a1ps", bufs=2,
                                          name="ps_a1")
                            nc.tensor.matmul(
                                ps[:], qhT[p0:p0 + 64, g, m * 128:(m + 1) * 128],
                                pTt[p0:p0 + 64, g, c * 512:(c + 1) * 512],
                                start=True, stop=True)
                            e = sp.tile([128, 512], F32, tag="e", bufs=3, name="e")
                            nc.scalar.activation(e[:], ps[:], ACTF.Exp)
                            e1 = sp.tile([128, 512], F32, tag="e1", bufs=4, name="e1")
                            nc.gpsimd.tensor_scalar_min(e1[:], e[:], 1.0)
                            nc.vector.scalar_tensor_tensor(
                                a1[:, m, c * 512:(c + 1) * 512], ps[:], 0.0, e1[:],
                                ALU.max, ALU.add)
                            yield
                        for n in range(m // 4, 2):
                            ps = ppm.tile([128, 512], F32, tag="mm", name="ps_sq")
                            nc.tensor.matmul(
                                ps[:], khT[p0:p0 + 64, g, m * 128:(m + 1) * 128],
                                qhT[p0:p0 + 64, g, n * 512:(n + 1) * 512],
                                start=True, stop=True)
                            dst = sqT[:, _sq_off(m, n * 512):_sq_off(m, n * 512) + 512]
                            if n == m // 4:
                                nc.vector.tensor_tensor(dst, ps[:], mask[:, m % 4, :], ALU.mult)
                            else:
                                nc.scalar.activation(dst, ps[:], ACTF.Copy)
                            yield
                        # U row-block i=m: a1/sqT blocks 0..m are all written
                        i = m
                        wblk = sp.tile([128, S], BF16, tag="wblk", bufs=6, name="wblk")
                        wblks.append(wblk)
                        dps = []
                        for c in range(2):
                            ps = ppm.tile([128, 512], F32, tag="mm", name="ps_u")
                            for mm in range(i + 1):
                                nc.tensor.matmul(
                                    ps[:], sqT[:, _sq_off(mm, i * 128):_sq_off(mm, i * 128) + 128],
                                    a1[:, mm, c * 512:(c + 1) * 512],
                                    start=(mm == 0), stop=(mm == i))
                            dp = sp.tile([128, 1], F32, tag="dp", bufs=6, name="dp")
                            nc.scalar.activation(
                                wblk[:, c * 512:(c + 1) * 512], ps[:], ACTF.Exp,
                                bias=wsh[:], scale=invidx[:, i:i + 1],
                                accum_out=dp[:])
                            dps.append(dp)
                            yield
                        nc.vector.tensor_tensor(denB[:, i:i + 1], dps[0][:], dps[1][:], ALU.add)
                        if m >= 2:
                            emit_w_transpose(m - 2)
                        if m == 1:
                            emit_a1t(0)
                        elif m == 2:
                            emit_a1t(1)
                        elif m == 3:
                            emit_a1t(2)
                    emit_w_transpose(NB - 2)
                    emit_w_transpose(NB - 1)
                    recden = sp.tile([128, NB], F32, tag="recden", name="recden")
                    nc.vector.reciprocal(recden[:], denB[:])
                    gsc = sp.tile([128, NB], F32, tag="gsc", bufs=2, name="gsc")
                    nc.vector.tensor_tensor(gsc[:], recden[:], invidx[:], ALU.mult)
                    st[h].append(gsc)

                def emit_final_tile(i):
                    # out[t-block i, :] = sum_g oT_g^T wc_g (all scales already
                    # folded into oT)
                    for c in (1, 0):
                        ftag = "mm" if (i + c) % 2 == 0 else "a1ps"
                        ps = ppm.tile([128, 512], F32, tag=ftag, name="ps_fin",
                                      bufs=(6 if ftag == "mm" else 2))
                        for g2 in range(2):
                            nc.tensor.matmul(
                                ps[:], oT[:, g2, i * 128:(i + 1) * 128],
                                wct[:, g2, c * 512:(c + 1) * 512],
                                start=(g2 == 0), stop=(g2 == 1))
                        ot = sp.tile([128, 512], F32, tag="ot", bufs=6, name="ot")
                        nc.scalar.activation(ot[:], ps[:], ACTF.Copy)
                        nc.scalar.dma_start(
                            out=out_d[i * 128:(i + 1) * 128, c * 512:(c + 1) * 512],
                            in_=ot[:])

                oNs = {}

                def emit_s2(h, inter=None):
                    def pull(k):
                        if inter is not None:
                            for _ in range(k):
                                if next(inter, "done") == "done":
                                    break
                    a1, sqT, wtT, a1Ts, emit_a1t, gsc = st.pop(h)
                    # S2T[s, t] = sum_j A1[s, j] W[t, j], tril(s<=t), emitted
                    # column-major (n = 256-wide t ranges) so consumption
                    # follows wtT production order, and each column pair's O
                    # chains run as soon as its s2T blocks land.
                    s2T = ap.tile([128, NB, S], BF16, tag="s2", bufs=1, name="s2T")
                    for nz in range(2):
                        for mz, w in ((4 * nz + 1, 128), (4 * nz + 2, 256),
                                      (4 * nz + 3, 384)):
                            # mask[:, 3, 0:384] is identically zero — zero-fill
                            # the never-computed gaps read by the O matmuls
                            nc.gpsimd.tensor_copy(
                                s2T[:, mz, nz * 512:nz * 512 + w], mask[:, 3, 0:w])
                    if h % 2 == 0:
                        oNs[h // 2] = sp.tile([128, NB, 128], BF16, tag="oN",
                                              bufs=2, name="oN")
                    oN = oNs[h // 2]
                    d0 = (h % 2) * 64

                    def emit_o_block(i):
                        # O[t, d] = gsc[t] * sum_{s<=t} S2T[s, t] vh[s, d]
                        ps = ppm.tile([128, 64], F32, tag="mm", name="ps_o")
                        for m in range(i + 1):
                            nc.tensor.matmul(
                                ps[:], s2T[:, m, i * 128:(i + 1) * 128],
                                vh[:, m, h * 64:(h + 1) * 64],
                                start=(m == 0), stop=(m == i))
                        nc.scalar.activation(oN[:, i, d0:d0 + 64], ps[:], ACTF.Copy,
                                             scale=gsc[:, i:i + 1])

                    for n in range(4):
                        for mpre in (2 * n + 3, 2 * n + 4):
                            if mpre < NB and mpre not in a1Ts:
                                emit_a1t(mpre)
                        for m in range(2 * n + 2):
                            a1T = a1Ts[m]
                            if m == 2 * n + 1:
                                pull(2)
                                ps = ppm.tile([128, 128], F32, tag="mm", name="ps_s2d")
                                for kp in range(NB // 2):
                                    nc.tensor.matmul(
                                        ps[:], a1T[:, 2 * kp:2 * kp + 2, :],
                                        wtT[:, 2 * kp:2 * kp + 2, m * 128:(m + 1) * 128],
                                        start=(kp == 0), stop=(kp == NB // 2 - 1),
                                        perf_mode=DROW)
                                nc.vector.tensor_tensor(
                                    s2T[:, m, m * 128:(m + 1) * 128], ps[:],
                                    mask[:, 0, 0:128], ALU.mult)
                            else:
                                pull(2)
                                ps = ppm.tile([128, 256], F32, tag="mm", name="ps_s2")
                                for kp in range(NB // 2):
                                    nc.tensor.matmul(
                                        ps[:], a1T[:, 2 * kp:2 * kp + 2, :],
                                        wtT[:, 2 * kp:2 * kp + 2, n * 256:(n + 1) * 256],
                                        start=(kp == 0), stop=(kp == NB // 2 - 1),
                                        perf_mode=DROW)
                                dst = s2T[:, m, n * 256:(n + 1) * 256]
                                if m == 2 * n:
                                    nc.vector.tensor_tensor(dst, ps[:], mask[:, 0, 0:256], ALU.mult)
                                else:
                                    nc.vector.tensor_copy(dst, ps[:])
                        def emit_tail(i):
                            # last pair: stack to [d, t] and run the output
                            # projection per O block so the tail drains
                            # pipelined instead of serially after all heads
                            tps = ppm.tile([128, 128], BF16, tag="mm",
                                           name="tpo")
                            nc.tensor.transpose(tps[:], oN[:, i, :], ident[:])
                            nc.vector.tensor_copy(
                                oT[:, 1, i * 128:(i + 1) * 128], tps[:])
                            emit_final_tile(i)

                        emit_o_block(2 * n)
                        pull(2)
                        emit_o_block(2 * n + 1)
                        pull(2)
                        if h == HG - 1 and n > 0:
                            # tail work lags one column so it never gates the
                            # O chains; the last pair drains after the loop
                            emit_tail(2 * n - 2)
                            emit_tail(2 * n - 1)
                    if h == HG - 1:
                        emit_tail(NB - 2)
                        emit_tail(NB - 1)
                    if h == 1:
                        # pair 0 done: stack to [d, t] via PE transpose for
                        # the K=128 output projection
                        oNf = oNs.pop(h // 2)
                        tps = ppm.tile([128, S], BF16, tag="mm", name="tpo")
                        for i in range(NB):
                            nc.tensor.transpose(
                                tps[:, i * 128:(i + 1) * 128], oNf[:, i, :], ident[:])
                        nc.scalar.activation(
                            oT[:, h // 2, :],
                            tps[:].rearrange("p (a b) -> p a b", a=NB), ACTF.Copy)

                # vh[s, d] = sum_c vT[c, s] wv[c, d] + wv_b[d], interleaved
                # with head 0's A1/SqT/U so PE has work while vT streams in
                gen0 = gen_head(0)
                for m in range(NB):
                    ps = ppm.tile([128, DL], F32, tag="mm", name="ps_vh")
                    for kb in range(NB):
                        nc.tensor.matmul(
                            ps[:], vTt[:, kb, m * 128:(m + 1) * 128], wvt[:, kb, :],
                            start=(kb == 0), stop=False)
                    nc.tensor.matmul(ps[:], ones1[:], wvb[:], start=False, stop=True)
                    nc.scalar.activation(vh[:, m, :], ps[:], ACTF.Copy)
                    for _ in range(3):
                        if next(gen0, "done") == "done":
                            break
                for _ in gen0:
                    pass
                if DEBUG:
                    nc.sync.dma_start(out=dbg["vh"].rearrange("p (a b) -> p a b", a=NB),
                                      in_=vh[:])
                for h in range(HG):
                    gen = gen_head(h + 1) if h + 1 < HG else None
                    emit_s2(h, inter=gen)
                    if gen is not None:
                        for _ in gen:
                            pass

            if DEBUG:
                nc.sync.dma_start(
                    out=dbg["oT"].rearrange("p (a b) -> p a b", a=HG), in_=oT[:])

            vp_cm.__exit__(None, None, None)

    nc.finalize()
    return nc


_CACHE = {}


def _get_program():
    if "nc" not in _CACHE:
        _CACHE["nc"] = _build_program()
    return _CACHE["nc"]


def _consts():
    if "consts" not in _CACHE:
        p_ = np.arange(128, dtype=np.float32)[:, None]
        c_ = np.arange(512, dtype=np.float32)[None, :]
        mask4 = np.stack(
            [(p_ + 128.0 * r <= c_) for r in range(4)]).astype(NPBF)
        ident = np.eye(128, dtype=np.float32).astype(NPBF)
        ident8 = np.eye(128, dtype=np.float32).astype(ml_dtypes.float8_e4m3)
        blk = np.arange(NB, dtype=np.float32)[None, :]
        invidx = (1.0 / (blk * 128.0 + p_ + 1.0)).astype(np.float32)
        ones1 = np.ones((1, 128), NPBF)
        _CACHE["consts"] = (mask4, ident, ident8, invidx, ones1)
    return _CACHE["consts"]


PROFILE = False
LAST_RESULTS = None


def kernel(v, k, q, p, wq_k, wq_b, wk_k, wk_b, wv_k, wv_b, wc_k, wc_b):
    global LAST_RESULTS
    nc = _get_program()
    mask4, ident, ident8, invidx, ones1 = _consts()

    qT = [np.ascontiguousarray(q[b].T).astype(NPBF) for b in range(B)]
    kT = [np.ascontiguousarray(k[b].T).astype(NPBF) for b in range(B)]
    vT = [np.ascontiguousarray(v[b].T).astype(NPBF) for b in range(B)]
    pT = [np.ascontiguousarray(p[b].T).astype(NPBF) for b in range(B)]
    wqc = wq_k.astype(NPBF)
    wkc = wk_k.astype(NPBF)
    wvc = wv_k.astype(NPBF)
    wcc = wc_k.astype(NPBF)

    in_maps = []
    for c in range(8):
        b, hg = c // 4, c % 4
        c0 = hg * DL
        wqb = np.ascontiguousarray(
            (wq_b[c0:c0 + DL].reshape(2, 128).T * NORM_D).astype(np.float32))
        wkb = np.ascontiguousarray(wk_b[c0:c0 + DL].reshape(2, 128).T.astype(np.float32))
        in_maps.append({
            "qT": qT[b], "kT": kT[b], "vT": vT[b],
            "pT": np.ascontiguousarray(pT[b][c0:c0 + DL]),
            "wq": np.ascontiguousarray(wqc[:, c0:c0 + DL]),
            "wk": np.ascontiguousarray(wkc[:, c0:c0 + DL]),
            "wv": np.ascontiguousarray(wvc[:, c0:c0 + DL]),
            "wc": np.ascontiguousarray(wcc[c0:c0 + DL, :]),
            "wqb": wqb, "wkb": wkb,
            "wvb": np.ascontiguousarray(wv_b[c0:c0 + DL].reshape(1, DL).astype(NPBF)),
            "ones1": ones1, "mask4": mask4, "ident": ident, "ident8": ident8,
            "invidx": invidx,
        })

    res = run_bass_kernel_spmd(
        nc, in_maps, core_ids=list(range(8)), trace=PROFILE)
    LAST_RESULTS = res

    out = np.zeros((B, S, DM), np.float32)
    for c in range(8):
        out[c // 4] += res.results[c]["out"]
    out += wc_b[None, None, :].astype(np.float32)
    return out



# revision 105
# speedup vs baseline: 1.0239x; 1.0239x over previous
"""Trainium2 Bass kernel for nn_MultiHeadAttention_75548474736720.

Linear-attention-style multi-head attention with causal prefix sums:
  qh/kh/vh = projections, ph = split_heads(p)
  A1 = elu(qh ph^T) + 1                       [t,s] per (b,h)
  U  = (tril(qh kh^T)/idx) @ A1 ; W = softmax(U)
  S2 = (tril(W A1^T)/idx) ; out = (S2 @ vh) reshaped @ wc + b

Sharding: 8 cores = (batch b in 0..1) x (head-group hg in 0..3, 4 heads each).
Each core computes its 4 heads end-to-end (wq/wk/wv column-sliced, wc
row-sliced) and returns a partial [S, Dm] output; host sums partials per batch.

Most matmuls run in bf16 (f32 PSUM accumulation); the S2 = W @ A1^T
contraction runs in fp8 with DoubleRow perf mode (2 k-tiles per
instruction at 0.5 cycles/row -> 4x the bf16 rate).  Measured end-to-end
error vs the f32 reference is ~1.4e-2 (gate 2e-2).  Key tricks:
  - W = exp(U/(t+1) - 10) cast to fp8e5: the constant shift keeps exp in
    e5m2 range (U/(t+1) in [-19, 19]) and cancels between S2 and den, so
    no per-row max pass is needed; A1^T cast to fp8e4 (A1 <= ~8)
  - softmax denominator via ACT accum_out (free with the exp pass, f32)
  - per-row 1/(t+1) scales folded into ACT scale APs (pre-exp and final)
  - W^T / A1^T via DMA-engine transposes (InstDmaTransposeAnt) + a DVE
    cast copy: no PE or ACT time spent on transposition
  - per-head generator emits A1[m] -> SqT[m] -> U[m] interleaved, pulled
    from inside head h-1's S2 phase: U is PE-heavy while S2 is DVE-heavy,
    so cross-head software pipelining keeps both engines fed
  - S2 emitted column-major matching wtT production order, with the O
    chains and (last pair) the output projection inlined per column
"""

import sys

sys.path.insert(0, "/opt/trn_rl_repo")

import ml_dtypes
import numpy as np

import concourse.bass as bass  # noqa: F401  (registers AP machinery)
import concourse.mybir as mybir
from concourse import bacc
from concourse.tile import TileContext
from concourse.bass_utils import run_bass_kernel_spmd

F32 = mybir.dt.float32
BF16 = mybir.dt.bfloat16
FP8 = mybir.dt.float8e4
FP8W = mybir.dt.float8e5
WSHIFT = -10.0
ACTF = mybir.ActivationFunctionType
ALU = mybir.AluOpType
AXL = mybir.AxisListType
DROW = mybir.MatmulPerfMode.DoubleRow
NPBF = ml_dtypes.bfloat16
LN16 = float(np.log(16.0))

B, S, DM, H = 2, 1024, 1024, 16
D = DM // H            # 64, head dim
HG = 4                 # heads per core
DL = HG * D            # 256, local dm slice
NB = S // 128          # 8 s-blocks
NORM_D = 0.125         # 1/sqrt(D)

# compact SqT layout: per s-block m, columns stored from t = 512*(m//4)
SQBASE = [0, 1024, 2048, 3072, 4096, 4608, 5120, 5632]  # total 6144

DEBUG = False


def _sq_off(m, t0):
    return SQBASE[m] + t0 - 512 * (m // 4)


def _build_program():
    nc = bacc.Bacc(None, target_bir_lowering=False)

    qT_in = nc.declare_dram_parameter("qT", [DM, S], BF16, isOutput=False)
    kT_in = nc.declare_dram_parameter("kT", [DM, S], BF16, isOutput=False)
    vT_in = nc.declare_dram_parameter("vT", [DM, S], BF16, isOutput=False)
    pT_in = nc.declare_dram_parameter("pT", [DL, S], BF16, isOutput=False)
    wq_in = nc.declare_dram_parameter("wq", [DM, DL], BF16, isOutput=False)
    wk_in = nc.declare_dram_parameter("wk", [DM, DL], BF16, isOutput=False)
    wv_in = nc.declare_dram_parameter("wv", [DM, DL], BF16, isOutput=False)
    wc_in = nc.declare_dram_parameter("wc", [DL, S], BF16, isOutput=False)
    wqb_in = nc.declare_dram_parameter("wqb", [128, 2], F32, isOutput=False)
    wkb_in = nc.declare_dram_parameter("wkb", [128, 2], F32, isOutput=False)
    wvb_in = nc.declare_dram_parameter("wvb", [1, DL], BF16, isOutput=False)
    ones_in = nc.declare_dram_parameter("ones1", [1, 128], BF16, isOutput=False)
    mask_in = nc.declare_dram_parameter("mask4", [4, 128, 512], BF16, isOutput=False)
    ident_in = nc.declare_dram_parameter("ident", [128, 128], BF16, isOutput=False)
    ident8_in = nc.declare_dram_parameter("ident8", [128, 128], FP8, isOutput=False)
    inv_in = nc.declare_dram_parameter("invidx", [128, NB], F32, isOutput=False)
    out_d = nc.declare_dram_parameter("out", [S, DM], F32, isOutput=True)
    dbg = {}
    if DEBUG:
        dbg["qhT"] = nc.declare_dram_parameter("d_qhT", [128, 2 * S], F32, isOutput=True)
        dbg["vh"] = nc.declare_dram_parameter("d_vh", [128, NB * DL], F32, isOutput=True)
        dbg["a1"] = nc.declare_dram_parameter("d_a1", [128, NB * S], F32, isOutput=True)
        dbg["sqT"] = nc.declare_dram_parameter("d_sqT", [128, 6144], F32, isOutput=True)
        dbg["wtT"] = nc.declare_dram_parameter("d_wtT", [128, NB * S], F32, isOutput=True)
        dbg["oT"] = nc.declare_dram_parameter("d_oT", [64, HG * S], F32, isOutput=True)
        dbg["den"] = nc.declare_dram_parameter("d_den", [128, NB], F32, isOutput=True)

    with TileContext(nc) as tc:
        with tc.tile_pool(name="persist", bufs=1) as cp, \
             tc.tile_pool(name="ppm", bufs=6, space="PSUM") as ppm:

            mask = cp.tile([128, 4, 512], BF16)
            ident = cp.tile([128, 128], BF16)
            ident8 = cp.tile([128, 128], FP8)
            invidx = cp.tile([128, NB], F32)
            neginv = cp.tile([128, NB], F32)
            wsh = cp.tile([128, 1], F32)
            dumt = cp.tile([128, 1], F32)
            wqb = cp.tile([128, 2], F32)
            wkb = cp.tile([128, 2], F32)
            wvb = cp.tile([1, DL], BF16)
            ones1 = cp.tile([1, 128], BF16)
            pTt = cp.tile([128, 2, S], BF16)
            qhT = cp.tile([128, 2, S], BF16)
            khT = cp.tile([128, 2, S], BF16)
            vh = cp.tile([128, NB, DL], BF16)
            oT = cp.tile([128, 2, S], BF16)
            # wc stored per head-pair: wct[:, g, :] = wc rows [g*128:(g+1)*128];
            # loaded up front so the output projection never waits on DMA
            wct = cp.tile([128, 2, S], BF16)

            # ---------------- projections ----------------
            # DMA issue on SP costs ~0.5us per descriptor, so the inputs the
            # first matmuls need go first, split 4-ways for queue parallelism;
            # constants (masks, wc, p) follow.  The v projection runs inside
            # the attention phase (interleaved with head 0's A1/SqT) so its
            # tiles live in a separate pool that outlives the q/k one.
            vp_cm = tc.tile_pool(name="vproj", bufs=1)
            vp = vp_cm.__enter__()
            wvt = vp.tile([128, NB, DL], BF16)
            vTt = vp.tile([128, NB, S], BF16)
            with tc.tile_pool(name="proj", bufs=1) as jp:
                wqt = jp.tile([128, NB, DL], BF16)
                wkt = jp.tile([128, NB, DL], BF16)
                qTt = jp.tile([128, NB, S], BF16)
                kTt = jp.tile([128, NB, S], BF16)
                for wt_, wsrc, xt_, xsrc in ((wqt, wq_in, qTt, qT_in),
                                             (wkt, wk_in, kTt, kT_in),
                                             (wvt, wv_in, vTt, vT_in)):
                    for q4 in range(4):
                        kb = 2 * q4
                        nc.sync.dma_start(
                            out=wt_[:, kb:kb + 2, :],
                            in_=wsrc[kb * 128:(kb + 2) * 128, :].rearrange(
                                "(a p) d -> p a d", p=128))
                        nc.sync.dma_start(
                            out=xt_[:, kb:kb + 2, :],
                            in_=xsrc[kb * 128:(kb + 2) * 128, :].rearrange(
                                "(a p) t -> p a t", p=128))
                    if wt_ is wqt:
                        nc.sync.dma_start(
                            out=pTt[:], in_=pT_in.rearrange("(g p) t -> p g t", p=128))
                        nc.sync.dma_start(out=wqb[:], in_=wqb_in[:])
                        nc.sync.dma_start(out=invidx[:], in_=inv_in[:])
                        nc.vector.tensor_scalar_mul(neginv[:], invidx[:], -1.0)
                        nc.gpsimd.memset(wsh[:], WSHIFT)
                        # dummy exp: pulls the ACT function-table load off the
                        # critical path (it otherwise serializes before the
                        # first real activation ~12us in)
                        nc.scalar.activation(dumt[:], wsh[:], ACTF.Exp)
                    elif wt_ is wkt:
                        nc.sync.dma_start(
                            out=mask[:], in_=mask_in.rearrange("r p c -> p r c"))
                        nc.sync.dma_start(out=ident[:], in_=ident_in[:])
                        nc.sync.dma_start(out=ident8[:], in_=ident8_in[:])
                        nc.sync.dma_start(out=wkb[:], in_=wkb_in[:])
                    else:
                        nc.sync.dma_start(out=wvb[:], in_=wvb_in[:])
                        nc.sync.dma_start(out=ones1[:], in_=ones_in[:])
                        nc.sync.dma_start(
                            out=wct[:], in_=wc_in.rearrange("(a p) t -> p a t", p=128))

                # qhT[dm, t] = sum_c wq[c, dm] qT[c, t]  (+bias, * 1/sqrt(D))
                # streamed: the kb loop is outermost so each accumulation step
                # depends only on the kb-block DMAs, overlapping PE with the
                # input load instead of waiting for the full tensor
                for wt_, xt_, dst, bias_t, scale in (
                    (wqt, qTt, qhT, wqb, NORM_D),
                    (wkt, kTt, khT, wkb, 1.0),
                ):
                    pss = [ppm.tile([128, 512], F32, tag="mm", name="ps_proj")
                           for _ in range(4)]
                    for kb in range(NB):
                        for g in range(2):
                            for n in range(2):
                                nc.tensor.matmul(
                                    pss[2 * g + n][:],
                                    wt_[:, kb, g * 128:(g + 1) * 128],
                                    xt_[:, kb, n * 512:(n + 1) * 512],
                                    start=(kb == 0), stop=(kb == NB - 1))
                    for g in range(2):
                        for n in range(2):
                            nc.scalar.activation(
                                dst[:, g, n * 512:(n + 1) * 512], pss[2 * g + n][:],
                                ACTF.Identity, bias=bias_t[:, g:g + 1], scale=scale)

                if DEBUG:
                    nc.sync.dma_start(out=dbg["qhT"].rearrange("p (a b) -> p a b", a=2),
                                      in_=qhT[:])

            # ---------------- attention (4 heads) ----------------
            # Pair-level software pipeline: A1/SqT for head h+1 are emitted
            # between U(h) and S2(h) so the in-order PE stream always has
            # independent matmuls to run while elementwise chains drain.
            with tc.tile_pool(name="attn", bufs=2) as ap, \
                 tc.tile_pool(name="scr", bufs=2) as sp:
                st = {}

                def gen_head(h):
                    """Generator: per m-block emits A1[m], SqT[m], then U[i=m]
                    (whose inputs are complete), yielding after each unit so
                    head h's whole front end can interleave into head h-1's
                    S2 phase: S2 is DVE/DMA-heavy while U is PE-heavy, so
                    overlapping them keeps both engines fed.

                    A1 = elu(x)+1 = min(exp(x), 1) + relu(x); exp is safe
                    unclamped (|x| <= ~8 here).  W = exp(U/(t+1) - 10) in
                    bf16 -> DMA-transposed -> cast to fp8e5; the constant
                    shift keeps exp within fp8e5 range (U/(t+1) <= ~19) and
                    cancels between S2 and den so gsc is unchanged."""
                    g, p0 = h // 2, (h % 2) * 64
                    a1 = ap.tile([128, NB, S], BF16, tag="a1", name="a1")
                    sqT = ap.tile([128, 6144], BF16, tag="sq", name="sqT")
                    wtT = ap.tile([128, NB, S], FP8W, tag="wtT", bufs=2, name="wtT")
                    a1Ts = {}
                    st[h] = [a1, sqT, wtT, a1Ts]
                    wblks = []

                    def emit_w_transpose(i):
                        wblk = wblks[i]
                        wtTb = sp.tile([128, NB, 128], BF16, tag="wtTb", bufs=3,
                                       name="wtTb")
                        nc.sync.dma_start(out=wtTb[:], in_=wblk[:], transpose=True)
                        nc.vector.tensor_copy(
                            wtT[:, :, i * 128:(i + 1) * 128], wtTb[:])

                    def emit_a1t(m):
                        a1Tb = sp.tile([128, NB, 128], BF16, tag="a1Tb", bufs=4,
                                       name="a1Tb")
                        a1T = sp.tile([128, NB, 128], FP8, tag="a1T", bufs=8,
                                      name="a1T")
                        a1Ts[m] = a1T
                        nc.sync.dma_start(out=a1Tb[:], in_=a1[:, m, :],
                                          transpose=True)
                        nc.vector.tensor_copy(a1T[:], a1Tb[:])

                    st[h].append(emit_a1t)
                    denB = sp.tile([128, NB], F32, tag="denB", name="denB")
                    for m in range(NB):
                        for c in range(2):
                            ps = ppm.tile([128, 512], F32, tag="a1ps", bufs=2,
                                          name="ps_a1")
                            nc.tensor.matmul(
                                ps[:], qhT[p0:p0 + 64, g, m * 128:(m + 1) * 128],
                                pTt[p0:p0 + 64, g, c * 512:(c + 1) * 512],
                                start=True, stop=True)
                            e = sp.tile([128, 512], F32, tag="e", bufs=3, name="e")
                            nc.scalar.activation(e[:], ps[:], ACTF.Exp)
                            e1 = sp.tile([128, 512], F32, tag="e1", bufs=4, name="e1")
                            nc.gpsimd.tensor_scalar_min(e1[:], e[:], 1.0)
                            nc.vector.scalar_tensor_tensor(
                                a1[:, m, c * 512:(c + 1) * 512], ps[:], 0.0, e1[:],
                                ALU.max, ALU.add)
                            yield
                        for n in range(m // 4, 2):
                            ps = ppm.tile([128, 512], F32, tag="mm", name="ps_sq")
                            nc.tensor.matmul(
                                ps[:], khT[p0:p0 + 64, g, m * 128:(m + 1) * 128],
                                qhT[p0:p0 + 64, g, n * 512:(n + 1) * 512],
                                start=True, stop=True)
                            dst = sqT[:, _sq_off(m, n * 512):_sq_off(m, n * 512) + 512]
                            if n == m // 4:
                                nc.vector.tensor_tensor(dst, ps[:], mask[:, m % 4, :], ALU.mult)
                            else:
                                nc.scalar.activation(dst, ps[:], ACTF.Copy)
                            yield
                        # U row-block i=m: a1/sqT blocks 0..m are all written
                        i = m
                        wblk = sp.tile([128, S], BF16, tag="wblk", bufs=6, name="wblk")
                        wblks.append(wblk)
                        dps = []
                        for c in range(2):
                            ps = ppm.tile([128, 512], F32, tag="mm", name="ps_u")
                            for mm in range(i + 1):
                                nc.tensor.matmul(
                                    ps[:], sqT[:, _sq_off(mm, i * 128):_sq_off(mm, i * 128) + 128],
                                    a1[:, mm, c * 512:(c + 1) * 512],
                                    start=(mm == 0), stop=(mm == i))
                            dp = sp.tile([128, 1], F32, tag="dp", bufs=6, name="dp")
                            nc.scalar.activation(
                                wblk[:, c * 512:(c + 1) * 512], ps[:], ACTF.Exp,
                                bias=wsh[:], scale=invidx[:, i:i + 1],
                                accum_out=dp[:])
                            dps.append(dp)
                            yield
                        nc.vector.tensor_tensor(denB[:, i:i + 1], dps[0][:], dps[1][:], ALU.add)
                        if m >= 2:
                            emit_w_transpose(m - 2)
                        if m == 1:
                            emit_a1t(0)
                        elif m == 2:
                            emit_a1t(1)
                        elif m == 3:
                            emit_a1t(2)
                    emit_w_transpose(NB - 2)
                    emit_w_transpose(NB - 1)
                    recden = sp.tile([128, NB], F32, tag="recden", name="recden")
                    nc.vector.reciprocal(recden[:], denB[:])
                    gsc = sp.tile([128, NB], F32, tag="gsc", bufs=2, name="gsc")
                    nc.vector.tensor_tensor(gsc[:], recden[:], invidx[:], ALU.mult)
                    st[h].append(gsc)

                def emit_final_tile(i):
                    # out[t-block i, :] = sum_g oT_g^T wc_g (all scales already
                    # folded into oT)
                    for c in (1, 0):
                        ftag = "mm" if (i + c) % 2 == 0 else "a1ps"
                        ps = ppm.tile([128, 512], F32, tag=ftag, name="ps_fin",
                                      bufs=(6 if ftag == "mm" else 2))
                        for g2 in range(2):
                            nc.tensor.matmul(
                                ps[:], oT[:, g2, i * 128:(i + 1) * 128],
                                wct[:, g2, c * 512:(c + 1) * 512],
                                start=(g2 == 0), stop=(g2 == 1))
                        ot = sp.tile([128, 512], F32, tag="ot", bufs=6, name="ot")
                        nc.scalar.activation(ot[:], ps[:], ACTF.Copy)
                        nc.sync.dma_start(
                            out=out_d[i * 128:(i + 1) * 128, c * 512:(c + 1) * 512],
                            in_=ot[:])

                oNs = {}

                def emit_s2(h, inter=None):
                    def pull(k):
                        if inter is not None:
                            for _ in range(k):
                                if next(inter, "done") == "done":
                                    break
                    a1, sqT, wtT, a1Ts, emit_a1t, gsc = st.pop(h)
                    # S2T[s, t] = sum_j A1[s, j] W[t, j], tril(s<=t), emitted
                    # column-major (n = 256-wide t ranges) so consumption
                    # follows wtT production order, and each column pair's O
                    # chains run as soon as its s2T blocks land.
                    s2T = ap.tile([128, NB, S], BF16, tag="s2", bufs=1, name="s2T")
                    # (the never-computed s2T gaps are not read: every O
                    # chain contracts only blocks m' <= i, which excludes all
                    # above-diagonal gap blocks in this column-major layout)
                    if h % 2 == 0:
                        oNs[h // 2] = sp.tile([128, NB, 128], BF16, tag="oN",
                                              bufs=2, name="oN")
                    oN = oNs[h // 2]
                    d0 = (h % 2) * 64

                    def emit_o_block(i):
                        # O[t, d] = gsc[t] * sum_{s<=t} S2T[s, t] vh[s, d]
                        ps = ppm.tile([128, 64], F32, tag="mm", name="ps_o")
                        for m in range(i + 1):
                            nc.tensor.matmul(
                                ps[:], s2T[:, m, i * 128:(i + 1) * 128],
                                vh[:, m, h * 64:(h + 1) * 64],
                                start=(m == 0), stop=(m == i))
                        nc.scalar.activation(oN[:, i, d0:d0 + 64], ps[:], ACTF.Copy,
                                             scale=gsc[:, i:i + 1])

                    for n in range(4):
                        for mpre in (2 * n + 3, 2 * n + 4):
                            if mpre < NB and mpre not in a1Ts:
                                emit_a1t(mpre)
                        for m in range(2 * n + 2):
                            a1T = a1Ts[m]
                            if m == 2 * n + 1:
                                pull(2)
                                ps = ppm.tile([128, 128], F32, tag="mm", name="ps_s2d")
                                for kp in range(NB // 2):
                                    nc.tensor.matmul(
                                        ps[:], a1T[:, 2 * kp:2 * kp + 2, :],
                                        wtT[:, 2 * kp:2 * kp + 2, m * 128:(m + 1) * 128],
                                        start=(kp == 0), stop=(kp == NB // 2 - 1),
                                        perf_mode=DROW)
                                nc.vector.tensor_tensor(
                                    s2T[:, m, m * 128:(m + 1) * 128], ps[:],
                                    mask[:, 0, 0:128], ALU.mult)
                            else:
                                pull(2)
                                ps = ppm.tile([128, 256], F32, tag="mm", name="ps_s2")
                                for kp in range(NB // 2):
                                    nc.tensor.matmul(
                                        ps[:], a1T[:, 2 * kp:2 * kp + 2, :],
                                        wtT[:, 2 * kp:2 * kp + 2, n * 256:(n + 1) * 256],
                                        start=(kp == 0), stop=(kp == NB // 2 - 1),
                                        perf_mode=DROW)
                                dst = s2T[:, m, n * 256:(n + 1) * 256]
                                if m == 2 * n:
                                    nc.vector.tensor_tensor(dst, ps[:], mask[:, 0, 0:256], ALU.mult)
                                else:
                                    nc.vector.tensor_copy(dst, ps[:])
                        def emit_tail(i):
                            # last pair: stack to [d, t] and run the output
                            # projection per O block so the tail drains
                            # pipelined instead of serially after all heads
                            tps = ppm.tile([128, 128], BF16, tag="mm",
                                           name="tpo")
                            nc.tensor.transpose(tps[:], oN[:, i, :], ident[:])
                            nc.vector.tensor_copy(
                                oT[:, 1, i * 128:(i + 1) * 128], tps[:])
                            emit_final_tile(i)

                        emit_o_block(2 * n)
                        pull(2)
                        emit_o_block(2 * n + 1)
                        pull(2)
                        if h == HG - 1 and n > 0:
                            # tail work lags one column so it never gates the
                            # O chains; the last pair drains after the loop
                            emit_tail(2 * n - 2)
                            emit_tail(2 * n - 1)
                    if h == HG - 1:
                        emit_tail(NB - 2)
                        emit_tail(NB - 1)
                    if h == 1:
                        # pair 0 done: stack to [d, t] via PE transpose for
                        # the K=128 output projection
                        oNf = oNs.pop(h // 2)
                        tps = ppm.tile([128, S], BF16, tag="mm", name="tpo")
                        for i in range(NB):
                            nc.tensor.transpose(
                                tps[:, i * 128:(i + 1) * 128], oNf[:, i, :], ident[:])
                        nc.scalar.activation(
                            oT[:, h // 2, :],
                            tps[:].rearrange("p (a b) -> p a b", a=NB), ACTF.Copy)

                # vh[s, d] = sum_c vT[c, s] wv[c, d] + wv_b[d], interleaved
                # with head 0's A1/SqT/U so PE has work while vT streams in
                gen0 = gen_head(0)
                for m in range(NB):
                    ps = ppm.tile([128, DL], F32, tag="mm", name="ps_vh")
                    for kb in range(NB):
                        nc.tensor.matmul(
                            ps[:], vTt[:, kb, m * 128:(m + 1) * 128], wvt[:, kb, :],
                            start=(kb == 0), stop=False)
                    nc.tensor.matmul(ps[:], ones1[:], wvb[:], start=False, stop=True)
                    nc.scalar.activation(vh[:, m, :], ps[:], ACTF.Copy)
                    for _ in range(3):
                        if next(gen0, "done") == "done":
                            break
                for _ in gen0:
                    pass
                if DEBUG:
                    nc.sync.dma_start(out=dbg["vh"].rearrange("p (a b) -> p a b", a=NB),
                                      in_=vh[:])
                for h in range(HG):
                    gen = gen_head(h + 1) if h + 1 < HG else None
                    emit_s2(h, inter=gen)
                    if gen is not None:
                        for _ in gen:
                            pass

            if DEBUG:
                nc.sync.dma_start(
                    out=dbg["oT"].rearrange("p (a b) -> p a b", a=HG), in_=oT[:])

            vp_cm.__exit__(None, None, None)

    nc.finalize()
    return nc


_CACHE = {}


def _get_program():
    if "nc" not in _CACHE:
        _CACHE["nc"] = _build_program()
    return _CACHE["nc"]


def _consts():
    if "consts" not in _CACHE:
        p_ = np.arange(128, dtype=np.float32)[:, None]
        c_ = np.arange(512, dtype=np.float32)[None, :]
        mask4 = np.stack(
            [(p_ + 128.0 * r <= c_) for r in range(4)]).astype(NPBF)
        ident = np.eye(128, dtype=np.float32).astype(NPBF)
        ident8 = np.eye(128, dtype=np.float32).astype(ml_dtypes.float8_e4m3)
        blk = np.arange(NB, dtype=np.float32)[None, :]
        invidx = (1.0 / (blk * 128.0 + p_ + 1.0)).astype(np.float32)
        ones1 = np.ones((1, 128), NPBF)
        _CACHE["consts"] = (mask4, ident, ident8, invidx, ones1)
    return _CACHE["consts"]


PROFILE = False
LAST_RESULTS = None


def kernel(v, k, q, p, wq_k, wq_b, wk_k, wk_b, wv_k, wv_b, wc_k, wc_b):
    global LAST_RESULTS
    nc = _get_program()
    mask4, ident, ident8, invidx, ones1 = _consts()

    qT = [np.ascontiguousarray(q[b].T).astype(NPBF) for b in range(B)]
    kT = [np.ascontiguousarray(k[b].T).astype(NPBF) for b in range(B)]
    vT = [np.ascontiguousarray(v[b].T).astype(NPBF) for b in range(B)]
    pT = [np.ascontiguousarray(p[b].T).astype(NPBF) for b in range(B)]
    wqc = wq_k.astype(NPBF)
    wkc = wk_k.astype(NPBF)
    wvc = wv_k.astype(NPBF)
    wcc = wc_k.astype(NPBF)

    in_maps = []
    for c in range(8):
        b, hg = c // 4, c % 4
        c0 = hg * DL
        wqb = np.ascontiguousarray(
            (wq_b[c0:c0 + DL].reshape(2, 128).T * NORM_D).astype(np.float32))
        wkb = np.ascontiguousarray(wk_b[c0:c0 + DL].reshape(2, 128).T.astype(np.float32))
        in_maps.append({
            "qT": qT[b], "kT": kT[b], "vT": vT[b],
            "pT": np.ascontiguousarray(pT[b][c0:c0 + DL]),
            "wq": np.ascontiguousarray(wqc[:, c0:c0 + DL]),
            "wk": np.ascontiguousarray(wkc[:, c0:c0 + DL]),
            "wv": np.ascontiguousarray(wvc[:, c0:c0 + DL]),
            "wc": np.ascontiguousarray(wcc[c0:c0 + DL, :]),
            "wqb": wqb, "wkb": wkb,
            "wvb": np.ascontiguousarray(wv_b[c0:c0 + DL].reshape(1, DL).astype(NPBF)),
            "ones1": ones1, "mask4": mask4, "ident": ident, "ident8": ident8,
            "invidx": invidx,
        })

    res = run_bass_kernel_spmd(
        nc, in_maps, core_ids=list(range(8)), trace=PROFILE)
    LAST_RESULTS = res

    out = np.zeros((B, S, DM), np.float32)
    for c in range(8):
        out[c // 4] += res.results[c]["out"]
    out += wc_b[None, None, :].astype(np.float32)
    return out

